# revision 12
# baseline (speedup 1.0000x reference)
"""MoE (top-2 routed GluMLP) Trainium2 kernel, expert x DFF-quarter sharded
over 8 NeuronCores.

Contract: kernel(**inputs) takes the FULL unsharded inputs
  x  [2, 2048, 1024] f32
  Wr [8, 1024] f32           router
  Wg [8, 4096, 1024] f32     gate proj per expert
  Wu [8, 4096, 1024] f32     up proj per expert
  Wd [8, 1024, 4096] f32     down proj per expert
and returns the FULL output [2, 2048, 1024] f32.

Strategy:
  - Routing (softmax + top-2 + renormalize) on host with jax on CPU using
    the exact ops of the reference, so selected experts and combine weights
    match the reference bit-for-bit.
  - Work unit = (expert, DFF-quarter) shard: 32 shards over 8 cores, 4 slots
    per core. Experts ranked by load; slot position k (width w_k) hosts the
    two experts ranked 2k/2k+1: their 4 quarters spread over cores 0-3 /
    4-7. Per-core token width sum(w_k) = sum of the odd-ranked loads, which
    is within ~1.4% of the theoretical minimum (vs ~2.7% for half-DFF
    pairing, whose two slot widths both pad to global maxima).
  - Each core computes unweighted partial GluMLP outputs for its four token
    slots:  part[:, t] = WdT[q] @ (relu(Wg[q] @ x_t) * (Wu[q] @ x_t))
    Host sums the four quarter-partials per expert (exact fp32), applies
    the top-2 combine weights, and scatter-adds into the full output.
  - Matmul operands in fp16 (same 10-bit mantissa as TF32), fp32 PSUM
    accumulation; host converts operands so rounding is exact.
  - Phase B streams tokens against Wg/Wu tiles (f-major h layout); x tiles
    are per-slot, JIT-streamed through a 3-buffer pool. Phase C keeps the
    core's WdT quarter-set resident in SBUF (prefetched behind phase B) and
    streams h as the moving operand against stationary Wd tiles.
  - ~92 warmup matmuls on scratch SBUF span the startup DMA window so the
    HAM clock gate (default 1.2GHz) is released before the real stream.

Env: MOE_MM_DT=f16 (default) | f32r (TF32, 2x DMA) | f32 (4x slower, exact).
"""

import math
import os
from contextlib import ExitStack

import numpy as np

import concourse.bass as bass
import concourse.tile as tile
from concourse import bacc, mybir
from concourse.bass_utils import run_bass_kernel_spmd

B, L, D, E, TOPK, DFF = 2, 2048, 1024, 8, 2, 4096
T = B * L
NCORES = 8
P = 128
NB = 512          # matmul moving-operand block (one PSUM bank of fp32 out)
DC = D // P       # 8 contraction chunks over D
DN = D // P       # 8 output chunks over D (phase C)
FM = DFF // P     # 32 chunks over DFF
QS = 4            # token slots per core (one DFF-quarter of one expert each)
QFM = FM // QS    # 8 f-chunks per quarter

F32 = mybir.dt.float32
F32R = mybir.dt.float32r
F16 = mybir.dt.float16

# Set to True (e.g. from test.py) to run with NTFF tracing and print HW time.
PROFILE = False
TRACE_CORES = None  # e.g. list(range(8)) to profile every core
LAST_EXEC_NS = None
# Matmul dtype for the big GluMLP matmuls.
MM_DT = {"f32": F32, "f32r": F32R, "f16": F16}[os.environ.get("MOE_MM_DT", "f16")]

# dc consumption order matched to slot-0 stripe arrival (sync: 3,6,0 behind
# wg0; scalar: 1,7,4 behind wu0; gpsimd: 2,5 -- its SWDGE moves ~276KB in
# ~4.5us, so it only gets two) so the first fm's accumulation chains start
# as soon as the first stripes land instead of waiting for the last.
DC_ORDER = [3, 1, 2, 6, 7, 0, 4, 5]
NWARM = 92   # warmup matmuls: span the whole startup DMA window (~7->13.7us,
             # ~27 cold at 107ns then warm at 56ns) so the HAM clock gate
             # (default 1.2GHz) is released AND STAYS released (an idle gap
             # >3.4us re-throttles) until the first x stripe lands


def _nblocks(tch):
    """Moving-dim blocks <=512, balanced: per-matmul cost is
    max(stream_cols/2.4GHz, ~100ns weight-load floor), so equal blocks beat
    512s-plus-tiny-tail (a tail below ~233 cols is pure LDW overhead)."""
    k = max(1, math.ceil(tch / NB))
    base, rem = divmod(tch, k)
    out, t = [], 0
    for i in range(k):
        nb = base + (1 if i < rem else 0)
        out.append((t, nb))
        t += nb
    return out


def _build_nc(widths):
    """Build the single-core Bass program (SPMD: all cores run the same NEFF).

    Four token slots of static widths (one expert-quarter each)."""
    nc = bacc.Bacc(
        "TRN2",
        target_bir_lowering=False,
        debug=False,
        enable_asserts=False,
        num_devices=NCORES,
    )
    mdt = MM_DT
    Tc = sum(widths)
    x_d = nc.dram_tensor("x", [P, DC, Tc], mdt, kind="ExternalInput").ap()
    # 32 weight tiles, slot-major: slot s owns tiles [s*QFM, (s+1)*QFM)
    wg_d = nc.dram_tensor("wg", [FM, P, DC, P], mdt, kind="ExternalInput").ap()
    wu_d = nc.dram_tensor("wu", [FM, P, DC, P], mdt, kind="ExternalInput").ap()
    # WdT resident layout: [p(f_inner), s*QFM + fc, dn, d_inner]
    wd_d = nc.dram_tensor("wd", [P, FM, DN, P], mdt, kind="ExternalInput").ap()
    # out in [dn, d_inner, t] layout; host transposes, sums quarter-partials,
    # and applies combine weights
    out_d = nc.dram_tensor("out", [DN, P, Tc], F16, kind="ExternalOutput").ap()

    with tile.TileContext(nc) as tc:
        with ExitStack() as ctx:
            _moe_body(ctx, tc, x_d, wg_d, wu_d, wd_d, out_d, widths)
    nc.compile()
    return nc


def _moe_body(ctx, tc, x_d, wg_d, wu_d, wd_d, out_d, widths):
    nc = tc.nc
    mdt = MM_DT
    offs = [sum(widths[:s]) for s in range(QS)]
    slots = list(zip(offs, widths))
    wmax = max(widths)

    # x is JIT per slot through 2 rotating buffers: slot s+2's stripes reuse
    # slot s's buffer, whose WAR releases exactly when slot s's chains end --
    # early enough that the transfer lands before slot s+2 begins. Slots >=2
    # ride gpsimd exclusively, so the WAR head-of-line block parks the
    # (otherwise idle) SWDGE queue, not the weight-critical fast queues.
    xpool = ctx.enter_context(tc.tile_pool(name="xpool", bufs=2))
    hpool = ctx.enter_context(tc.tile_pool(name="hpool", bufs=1))
    wgupool = ctx.enter_context(tc.tile_pool(name="wgupool", bufs=5))
    wdpool = ctx.enter_context(tc.tile_pool(name="wdpool", bufs=1))
    gopool = ctx.enter_context(tc.tile_pool(name="gopool", bufs=3))
    scrpool = ctx.enter_context(tc.tile_pool(name="scrpool", bufs=1))
    # One shared PSUM pool: phase B (ps_g/ps_u) and phase C (per-dn out)
    # don't overlap in time, so both get all 8 banks.
    psP = ctx.enter_context(tc.tile_pool(name="psP", bufs=8, space="PSUM"))

    # Resident WdT: all four quarter tile sets, streamed in behind phase B.
    wd_sb = wdpool.tile([P, FM, DN, P], mdt, tag="wd")
    h_all = hpool.tile([P, QFM, sum(widths)], mdt, tag="h")
    wscr = scrpool.tile([P, P], mdt, tag="wscr")

    s0_q = {3: nc.sync, 6: nc.sync, 0: nc.sync,
            1: nc.scalar, 7: nc.scalar, 4: nc.scalar,
            2: nc.gpsimd, 5: nc.gpsimd}
    s1_q = {3: nc.sync, 6: nc.sync,
            1: nc.scalar, 7: nc.scalar,
            2: nc.gpsimd, 5: nc.gpsimd, 0: nc.gpsimd, 4: nc.gpsimd}

    def emit_x(s, x_tiles):
        off, W = slots[s]
        xt_sb = xpool.tile([P, DC, wmax], mdt, tag="x", name=f"x_s{s}")
        x_tiles[s] = xt_sb
        for dc in DC_ORDER:
            # slot-0 stripes split across all three queues (latency-
            # critical); later slots ride gpsimd (slow SWDGE but off the
            # weight-critical fast queues), with sync/scalar helping on two
            # stripes each to keep arrival ahead of consumption.
            if s == 0:
                q = s0_q[dc]
            elif s == 1:
                q = s1_q[dc]
            else:
                q = nc.gpsimd
            q.dma_start(
                out=xt_sb[:, dc, :W], in_=x_d[:, dc, off : off + W]
            )

    # ---- Phase A: streamed loads, ordered for fastest matmul start ----
    nc.vector.memset(wscr, 0.0)
    pre = []
    wg_p = wgupool.tile([P, DC, P], mdt, tag="wg", name="wg_pre0")
    nc.sync.dma_start(out=wg_p, in_=wg_d[0])
    wu_p = wgupool.tile([P, DC, P], mdt, tag="wu", name="wu_pre0")
    nc.scalar.dma_start(out=wu_p, in_=wu_d[0])
    pre.append((wg_p, wu_p))
    # Warmup matmuls on scratch SBUF: release the HAM clock gate during the
    # startup DMA window (see NWARM).
    for w in range(NWARM):
        ps_w = psP.tile([P, NB], F32, tag="ps", name=f"warm{w}")
        nc.tensor.matmul(ps_w[:, :P], lhsT=wscr, rhs=wscr, start=True, stop=True)
    x_tiles = {}
    emit_x(0, x_tiles)
    for fm in range(1, 4):
        wg_p = wgupool.tile([P, DC, P], mdt, tag="wg", name=f"wg_pre{fm}")
        nc.sync.dma_start(out=wg_p, in_=wg_d[fm])
        wu_p = wgupool.tile([P, DC, P], mdt, tag="wu", name=f"wu_pre{fm}")
        nc.scalar.dma_start(out=wu_p, in_=wu_d[fm])
        pre.append((wg_p, wu_p))
    emit_x(1, x_tiles)

    # ---- Phase B: h[f, t] = relu(g) * u, f-major layout, slot-major ----
    for s, (off, W) in enumerate(slots):
        if s + 2 < QS:
            emit_x(s + 2, x_tiles)
        x_sb = x_tiles[s]
        blocks = _nblocks(W)
        for fl in range(QFM):
            fmg = s * QFM + fl
            if fmg < len(pre):
                wg_sb, wu_sb = pre[fmg]
            else:
                wg_sb = wgupool.tile([P, DC, P], mdt, tag="wg")
                nc.sync.dma_start(out=wg_sb, in_=wg_d[fmg])
                wu_sb = wgupool.tile([P, DC, P], mdt, tag="wu")
                nc.scalar.dma_start(out=wu_sb, in_=wu_d[fmg])
            if 8 <= fmg < 24:
                # Wd prefetch: 16 slabs of 512KB, deferred past the latency-
                # critical early window and done well ahead of phase C.
                fc2 = fmg - 8
                eng = [nc.sync, nc.scalar][fc2 % 2]
                eng.dma_start(
                    out=wd_sb[:, 2 * fc2 : 2 * fc2 + 2],
                    in_=wd_d[:, 2 * fc2 : 2 * fc2 + 2],
                )
            for nb0, nbl in blocks:
                ps_g = psP.tile([P, NB], F32, tag="ps")
                ps_u = psP.tile([P, NB], F32, tag="ps")
                for i, dc in enumerate(DC_ORDER):
                    nc.tensor.matmul(
                        ps_g[:, :nbl],
                        lhsT=wg_sb[:, dc, :],
                        rhs=x_sb[:, dc, nb0 : nb0 + nbl],
                        start=(i == 0),
                        stop=(i == DC - 1),
                    )
                for i, dc in enumerate(DC_ORDER):
                    nc.tensor.matmul(
                        ps_u[:, :nbl],
                        lhsT=wu_sb[:, dc, :],
                        rhs=x_sb[:, dc, nb0 : nb0 + nbl],
                        start=(i == 0),
                        stop=(i == DC - 1),
                    )
                # Both drain ops on the vector engine: the scalar engine's
                # stream carries pool-paced DMA triggers whose waits would
                # head-of-line-block a relu placed behind them.
                g_sb = gopool.tile([P, NB], F32, tag="g")
                nc.vector.tensor_scalar_max(g_sb[:, :nbl], ps_g[:, :nbl], 0.0)
                nc.vector.tensor_mul(
                    h_all[:, fl, off + nb0 : off + nb0 + nbl],
                    g_sb[:, :nbl],
                    ps_u[:, :nbl],
                )

    # ---- Phase C: part[d, t] = WdT.T @ h, Wd stationary / h moving ----
    # Per (slot, token block, dn): one PSUM bank accumulates 8 back-to-back
    # matmuls; drains and output DMAs rotate across engines / queues. The
    # slow SWDGE (gpsimd) queue only gets every 5th output DMA (the doubled
    # output volume would otherwise outlast phase C on it), and none of the
    # last 2 blocks', so its end-of-program queue drain doesn't extend the
    # tail. The final block is the narrowest and its last drain runs on
    # scalar (the vector queue carries a semaphore backlog at the end).
    qi = 0
    units = []
    for s in reversed(range(QS)):
        off, W = slots[s]
        for nb0, nbl in _nblocks(W):
            units.append((s, off, nb0, nbl))
    nunits = len(units)
    for ui, (s, off, nb0, nbl) in enumerate(units):
        late = ui >= nunits - 2
        for dn in range(DN):
            ps_o = psP.tile([P, NB], F32, tag="ps")
            for fl in range(QFM):
                nc.tensor.matmul(
                    ps_o[:, :nbl],
                    lhsT=wd_sb[:, s * QFM + fl, dn, :],
                    rhs=h_all[:, fl, off + nb0 : off + nb0 + nbl],
                    start=(fl == 0),
                    stop=(fl == QFM - 1),
                )
            o_sb = gopool.tile([P, NB], F16, tag="o")
            # each unit's output DMA rides the queue of the engine that
            # drains it (scalar drain -> scalar queue, vector drain -> sync
            # queue): the DMA's read-after-drain wait is then satisfied in
            # program order and never head-of-line-blocks another engine's
            # drains. The slow SWDGE (gpsimd) queue gets every 5th unit,
            # none in the last 2 blocks.
            par = (dn % 2 == 0) if ui < nunits - 1 else (dn % 2 == 1)
            if par:
                nc.scalar.activation(
                    out=o_sb[:, :nbl],
                    in_=ps_o[:, :nbl],
                    func=mybir.ActivationFunctionType.Copy,
                )
            else:
                nc.vector.tensor_scalar_mul(o_sb[:, :nbl], ps_o[:, :nbl], 1.0)
            if not late and qi % 5 == 4:
                q = nc.gpsimd
            else:
                q = nc.scalar if par else nc.sync
            q.dma_start(
                out=out_d[dn, :, off + nb0 : off + nb0 + nbl],
                in_=o_sb[:, :nbl],
            )
            qi += 1


_NC_CACHE: dict = {}


def _get_nc(widths):
    if widths not in _NC_CACHE:
        _NC_CACHE[widths] = _build_nc(widths)
    return _NC_CACHE[widths]


def _round_tf32(a):
    """Round-to-nearest-even fp32 -> TF32 (10-bit mantissa), as np.float32."""
    u = a.astype(np.float32).view(np.uint32).astype(np.uint64)
    lsb = (u >> 13) & 1
    r = (u + 0x0FFF + lsb) & 0xFFFFE000
    return r.astype(np.uint32).view(np.float32)


def _mm_round(a):
    """Convert a host array to the dtype/value the device matmuls consume."""
    if MM_DT is F32R:
        return _round_tf32(a)
    if MM_DT is F16:
        return np.ascontiguousarray(a, dtype=np.float16)
    return np.ascontiguousarray(a, dtype=np.float32)


def _route_host(x, Wr):
    """Reference-identical routing on host (jax on CPU, same ops as reference).

    Returns (k_ids [T, K] int, k_w [T, K] f32).
    """
    import jax
    import jax.numpy as jnp

    cpu = jax.devices("cpu")[0]
    with jax.default_device(cpu):
        xt = jnp.asarray(x.reshape(T, D))
        logits = jnp.einsum("td,ed->te", xt, jnp.asarray(Wr))
        scores = jax.nn.softmax(logits, axis=-1)
        k_scores, k_ids = jax.lax.top_k(scores, TOPK)
        eps = jnp.finfo(x.dtype).eps
        k_w = k_scores / (k_scores.sum(axis=-1, keepdims=True) + eps)
        return np.asarray(k_ids), np.asarray(k_w)


def _prep_weights(Wg, Wu, Wd):
    """Per-expert weight tensors in device layouts (contiguous, rounded)."""
    wg_r, wu_r, wd_r = [], [], []
    for e in range(len(Wg)):
        # Wg[e]: [DFF, D]; device wants [fm, p(d_inner), dc, f_inner]
        wgt = Wg[e].T.reshape(DC, P, FM, P).transpose(2, 1, 0, 3)
        wut = Wu[e].T.reshape(DC, P, FM, P).transpose(2, 1, 0, 3)
        # Wd[e]: [D, DFF]; device wants [p(f_inner), fc, dn, d_inner]
        wdt = Wd[e].reshape(DN, P, FM, P).transpose(3, 2, 0, 1)
        wg_r.append(_mm_round(np.ascontiguousarray(wgt, dtype=np.float32)))
        wu_r.append(_mm_round(np.ascontiguousarray(wut, dtype=np.float32)))
        wd_r.append(_mm_round(np.ascontiguousarray(wdt, dtype=np.float32)))
    return wg_r, wu_r, wd_r


def kernel(x, Wr, Wg, Wu, Wd):
    global LAST_EXEC_NS
    x = np.asarray(x, dtype=np.float32)
    Wr = np.asarray(Wr, dtype=np.float32)
    Wg = np.asarray(Wg, dtype=np.float32)
    Wu = np.asarray(Wu, dtype=np.float32)
    Wd = np.asarray(Wd, dtype=np.float32)

    k_ids, k_w = _route_host(x, Wr)
    xt = x.reshape(T, D)

    # Gather per-expert token lists (each token appears once per selected expert).
    idx_lists, w_lists = [], []
    for e in range(E):
        tmask = k_ids == e                       # [T, K]
        tok = np.nonzero(tmask.any(axis=1))[0]   # unique tokens routed to e
        wvals = (k_w * tmask).sum(axis=1)[tok].astype(np.float32)
        idx_lists.append(tok)
        w_lists.append(wvals)

    loads = np.array([len(t) for t in idx_lists])
    # Rank experts by load; slot position k hosts experts ranked 2k (cores
    # 0-3) and 2k+1 (cores 4-7), one DFF-quarter per core, so slot width k
    # only pads to the rank-2k load instead of the global max.
    order = [int(e) for e in np.argsort(-loads, kind="stable")]
    widths = tuple(
        max(P, ((int(loads[order[2 * k]]) + 7) // 8) * 8) for k in range(QS)
    )
    offs = [sum(widths[:s]) for s in range(QS)]
    Tc = sum(widths)
    assert Tc <= 4600 and max(widths) <= 1400, f"imbalanced routing ({widths})"

    wg_r, wu_r, wd_r = _prep_weights(Wg, Wu, Wd)

    in_maps = []
    for c in range(NCORES):
        q = c % 4
        xg = np.zeros((Tc, D), dtype=np.float32)
        wg_t, wu_t, wd_t = [], [], []
        for k in range(QS):
            e = order[2 * k] if c < 4 else order[2 * k + 1]
            xg[offs[k] : offs[k] + loads[e]] = xt[idx_lists[e]]
            sl = slice(q * QFM, (q + 1) * QFM)
            wg_t.append(wg_r[e][sl])
            wu_t.append(wu_r[e][sl])
            wd_t.append(wd_r[e][:, sl])
        xg_r = _mm_round(
            np.ascontiguousarray(
                xg.T.reshape(DC, P, Tc).transpose(1, 0, 2), dtype=np.float32
            )
        )
        in_maps.append(
            {
                "x": xg_r,
                "wg": np.ascontiguousarray(np.concatenate(wg_t, axis=0)),
                "wu": np.ascontiguousarray(np.concatenate(wu_t, axis=0)),
                "wd": np.ascontiguousarray(np.concatenate(wd_t, axis=1)),
            }
        )

    nc = _get_nc(widths)
    core_ids = list(range(NCORES))
    if PROFILE:
        res = _run_profiled(nc, in_maps, core_ids)
        LAST_EXEC_NS = res.exec_time_ns
        results = res.results
    else:
        results = run_bass_kernel_spmd(nc, in_maps, core_ids).results

    out = np.zeros((T, D), dtype=np.float32)
    outs = [results[c]["out"].reshape(D, Tc).astype(np.float32) for c in range(NCORES)]
    for k in range(QS):
        for group, e in ((range(0, 4), order[2 * k]), (range(4, 8), order[2 * k + 1])):
            # the four quarter-cores' partials sum to the full GluMLP output
            # (exact fp32 adds)
            oe = sum(outs[c][:, offs[k] : offs[k] + loads[e]] for c in group)
            out[idx_lists[e]] += w_lists[e][:, None] * oe.T
    return out.reshape(B, L, D)


def _run_profiled(nc, in_maps, core_ids):
    """run_bass_kernel_spmd with trace=True, providing the NTFF hook that the
    agent image's antenv stub lacks, and skipping the artifact upload."""
    import sys
    import tempfile
    import types

    import concourse.bass_utils as bu

    if "antenv.axon_hooks" not in sys.modules:
        from trn_agent_boot.trn_boot import _ntff_profile_via_ctypes

        hook = _ntff_profile_via_ctypes("/opt/axon/libaxon_pjrt.so")
        mod = types.ModuleType("antenv.axon_hooks")
        mod.get_axon_ntff_profile_hook = lambda: hook
        mod.set_axon_ntff_profile_hook = lambda h: None
        sys.modules["antenv.axon_hooks"] = mod

    orig_upload = bu.upload_artifacts
    bu.upload_artifacts = lambda tmpdir: ""
    try:
        return run_bass_kernel_spmd(
            nc,
            in_maps,
            core_ids,
            trace=True,
            trace_cores=TRACE_CORES,
            tmpdir=tempfile.mkdtemp(prefix="moe_ntff_"),
        )
    finally:
        bu.upload_artifacts = orig_upload


if __name__ == "__main__":
    # smoke test with random data (no reference comparison)
    rng = np.random.default_rng(0)
    ins = {
        "x": rng.standard_normal((B, L, D), dtype=np.float32),
        "Wr": (rng.standard_normal((E, D)) * 0.02).astype(np.float32),
        "Wg": (rng.standard_normal((E, DFF, D)) * 0.02).astype(np.float32),
        "Wu": (rng.standard_normal((E, DFF, D)) * 0.02).astype(np.float32),
        "Wd": (rng.standard_normal((E, D, DFF)) * 0.02).astype(np.float32),
    }
    out = kernel(**ins)
    print("out", out.shape, out.dtype, float(np.abs(out).max()))


# revision 13
# speedup vs baseline: 1.0077x; 1.0077x over previous
"""MoE (top-2 routed GluMLP) Trainium2 kernel, expert x DFF-quarter sharded
over 8 NeuronCores.

Contract: kernel(**inputs) takes the FULL unsharded inputs
  x  [2, 2048, 1024] f32
  Wr [8, 1024] f32           router
  Wg [8, 4096, 1024] f32     gate proj per expert
  Wu [8, 4096, 1024] f32     up proj per expert
  Wd [8, 1024, 4096] f32     down proj per expert
and returns the FULL output [2, 2048, 1024] f32.

Strategy:
  - Routing (softmax + top-2 + renormalize) on host with jax on CPU using
    the exact ops of the reference, so selected experts and combine weights
    match the reference bit-for-bit.
  - Work unit = (expert, DFF-quarter) shard: 32 shards over 8 cores, 4 slots
    per core. Experts ranked by load; slot position k (width w_k) hosts the
    two experts ranked 2k/2k+1: their 4 quarters spread over cores 0-3 /
    4-7. Per-core token width sum(w_k) = sum of the odd-ranked loads, which
    is within ~1.4% of the theoretical minimum (vs ~2.7% for half-DFF
    pairing, whose two slot widths both pad to global maxima).
  - Each core computes unweighted partial GluMLP outputs for its four token
    slots:  part[:, t] = WdT[q] @ (relu(Wg[q] @ x_t) * (Wu[q] @ x_t))
    Host sums the four quarter-partials per expert (exact fp32), applies
    the top-2 combine weights, and scatter-adds into the full output.
  - Matmul operands in fp16 (same 10-bit mantissa as TF32), fp32 PSUM
    accumulation; host converts operands so rounding is exact.
  - Phase B streams tokens against Wg/Wu tiles (f-major h layout); x tiles
    are per-slot, JIT-streamed through a 3-buffer pool. Phase C keeps the
    core's WdT quarter-set resident in SBUF (prefetched behind phase B) and
    streams h as the moving operand against stationary Wd tiles.
  - ~92 warmup matmuls on scratch SBUF span the startup DMA window so the
    HAM clock gate (default 1.2GHz) is released before the real stream.

Env: MOE_MM_DT=f16 (default) | f32r (TF32, 2x DMA) | f32 (4x slower, exact).
"""

import math
import os
from contextlib import ExitStack

import numpy as np

import concourse.bass as bass
import concourse.tile as tile
from concourse import bacc, mybir
from concourse.bass_utils import run_bass_kernel_spmd

B, L, D, E, TOPK, DFF = 2, 2048, 1024, 8, 2, 4096
T = B * L
NCORES = 8
P = 128
NB = 512          # matmul moving-operand block (one PSUM bank of fp32 out)
DC = D // P       # 8 contraction chunks over D
DN = D // P       # 8 output chunks over D (phase C)
FM = DFF // P     # 32 chunks over DFF
QS = 4            # token slots per core (one DFF-quarter of one expert each)
QFM = FM // QS    # 8 f-chunks per quarter

F32 = mybir.dt.float32
F32R = mybir.dt.float32r
F16 = mybir.dt.float16

# Set to True (e.g. from test.py) to run with NTFF tracing and print HW time.
PROFILE = False
TRACE_CORES = None  # e.g. list(range(8)) to profile every core
LAST_EXEC_NS = None
# Matmul dtype for the big GluMLP matmuls.
MM_DT = {"f32": F32, "f32r": F32R, "f16": F16}[os.environ.get("MOE_MM_DT", "f16")]

# dc consumption order matched to stripe arrival (sync: 3,6 behind wg0;
# scalar: 1,7 behind wu0; gpsimd: 2,5,0,4 -- SWDGE is fastest early) so the
# first fm's accumulation chain starts as soon as the first stripes land
# instead of waiting for the last.
DC_ORDER = [2, 5, 3, 1, 0, 6, 7, 4]
NWARM = 92   # warmup matmuls: span the whole startup DMA window (~7->13.7us,
             # ~27 cold at 107ns then warm at 56ns) so the HAM clock gate
             # (default 1.2GHz) is released AND STAYS released (an idle gap
             # >3.4us re-throttles) until the first x stripe lands


def _nblocks(tch):
    """Moving-dim blocks <=512, balanced: per-matmul cost is
    max(stream_cols/2.4GHz, ~100ns weight-load floor), so equal blocks beat
    512s-plus-tiny-tail (a tail below ~233 cols is pure LDW overhead)."""
    k = max(1, math.ceil(tch / NB))
    base, rem = divmod(tch, k)
    out, t = [], 0
    for i in range(k):
        nb = base + (1 if i < rem else 0)
        out.append((t, nb))
        t += nb
    return out


def _build_nc(widths):
    """Build the single-core Bass program (SPMD: all cores run the same NEFF).

    Four token slots of static widths (one expert-quarter each)."""
    nc = bacc.Bacc(
        "TRN2",
        target_bir_lowering=False,
        debug=False,
        enable_asserts=False,
        num_devices=NCORES,
    )
    mdt = MM_DT
    Tc = sum(widths)
    x_d = nc.dram_tensor("x", [P, DC, Tc], mdt, kind="ExternalInput").ap()
    # 32 weight tiles, slot-major: slot s owns tiles [s*QFM, (s+1)*QFM)
    wg_d = nc.dram_tensor("wg", [FM, P, DC, P], mdt, kind="ExternalInput").ap()
    wu_d = nc.dram_tensor("wu", [FM, P, DC, P], mdt, kind="ExternalInput").ap()
    # WdT resident layout: [p(f_inner), s*QFM + fc, dn, d_inner]
    wd_d = nc.dram_tensor("wd", [P, FM, DN, P], mdt, kind="ExternalInput").ap()
    # out in [dn, d_inner, t] layout; host transposes, sums quarter-partials,
    # and applies combine weights
    out_d = nc.dram_tensor("out", [DN, P, Tc], F16, kind="ExternalOutput").ap()

    with tile.TileContext(nc) as tc:
        with ExitStack() as ctx:
            _moe_body(ctx, tc, x_d, wg_d, wu_d, wd_d, out_d, widths)
    nc.compile()
    return nc


def _moe_body(ctx, tc, x_d, wg_d, wu_d, wd_d, out_d, widths):
    nc = tc.nc
    mdt = MM_DT
    offs = [sum(widths[:s]) for s in range(QS)]
    slots = list(zip(offs, widths))
    wmax = max(widths)

    # x is JIT per slot through 2 rotating buffers: slot s+2's stripes reuse
    # slot s's buffer, whose WAR releases exactly when slot s's chains end --
    # early enough that the transfer lands before slot s+2 begins. Slots >=2
    # ride gpsimd exclusively, so the WAR head-of-line block parks the
    # (otherwise idle) SWDGE queue, not the weight-critical fast queues.
    xpool = ctx.enter_context(tc.tile_pool(name="xpool", bufs=2))
    hpool = ctx.enter_context(tc.tile_pool(name="hpool", bufs=1))
    wgupool = ctx.enter_context(tc.tile_pool(name="wgupool", bufs=5))
    wdpool = ctx.enter_context(tc.tile_pool(name="wdpool", bufs=1))
    gopool = ctx.enter_context(tc.tile_pool(name="gopool", bufs=3))
    scrpool = ctx.enter_context(tc.tile_pool(name="scrpool", bufs=1))
    # One shared PSUM pool: phase B (ps_g/ps_u) and phase C (per-dn out)
    # don't overlap in time, so both get all 8 banks.
    psP = ctx.enter_context(tc.tile_pool(name="psP", bufs=8, space="PSUM"))

    # Resident WdT: all four quarter tile sets, streamed in behind phase B.
    wd_sb = wdpool.tile([P, FM, DN, P], mdt, tag="wd")
    h_all = hpool.tile([P, QFM, sum(widths)], mdt, tag="h")
    wscr = scrpool.tile([P, P], mdt, tag="wscr")

    stripe_q = {3: nc.sync, 6: nc.sync,
                1: nc.scalar, 7: nc.scalar,
                2: nc.gpsimd, 5: nc.gpsimd, 0: nc.gpsimd, 4: nc.gpsimd}

    def emit_x(s, x_tiles):
        off, W = slots[s]
        xt_sb = xpool.tile([P, DC, wmax], mdt, tag="x", name=f"x_s{s}")
        x_tiles[s] = xt_sb
        for dc in DC_ORDER:
            # slot-0 stripes split across all three queues (latency-
            # critical); later slots ride gpsimd (slow SWDGE but off the
            # weight-critical fast queues), with sync/scalar helping on two
            # stripes each to keep arrival ahead of consumption.
            if s <= 1:
                q = stripe_q[dc]
            else:
                q = nc.gpsimd
            q.dma_start(
                out=xt_sb[:, dc, :W], in_=x_d[:, dc, off : off + W]
            )

    # ---- Phase A: streamed loads, ordered for fastest matmul start ----
    nc.vector.memset(wscr, 0.0)
    pre = []
    wg_p = wgupool.tile([P, DC, P], mdt, tag="wg", name="wg_pre0")
    nc.sync.dma_start(out=wg_p, in_=wg_d[0])
    wu_p = wgupool.tile([P, DC, P], mdt, tag="wu", name="wu_pre0")
    nc.scalar.dma_start(out=wu_p, in_=wu_d[0])
    pre.append((wg_p, wu_p))
    # Warmup matmuls on scratch SBUF: release the HAM clock gate during the
    # startup DMA window (see NWARM).
    for w in range(NWARM):
        ps_w = psP.tile([P, NB], F32, tag="ps", name=f"warm{w}")
        nc.tensor.matmul(ps_w[:, :P], lhsT=wscr, rhs=wscr, start=True, stop=True)
    x_tiles = {}
    emit_x(0, x_tiles)
    for fm in range(1, 4):
        wg_p = wgupool.tile([P, DC, P], mdt, tag="wg", name=f"wg_pre{fm}")
        nc.sync.dma_start(out=wg_p, in_=wg_d[fm])
        wu_p = wgupool.tile([P, DC, P], mdt, tag="wu", name=f"wu_pre{fm}")
        nc.scalar.dma_start(out=wu_p, in_=wu_d[fm])
        pre.append((wg_p, wu_p))
    emit_x(1, x_tiles)

    # ---- Phase B: h[f, t] = relu(g) * u, f-major layout, slot-major ----
    for s, (off, W) in enumerate(slots):
        if s + 2 < QS:
            emit_x(s + 2, x_tiles)
        x_sb = x_tiles[s]
        blocks = _nblocks(W)
        for fl in range(QFM):
            fmg = s * QFM + fl
            if fmg < len(pre):
                wg_sb, wu_sb = pre[fmg]
            else:
                wg_sb = wgupool.tile([P, DC, P], mdt, tag="wg")
                nc.sync.dma_start(out=wg_sb, in_=wg_d[fmg])
                wu_sb = wgupool.tile([P, DC, P], mdt, tag="wu")
                nc.scalar.dma_start(out=wu_sb, in_=wu_d[fmg])
            if 8 <= fmg < 24:
                # Wd prefetch: 16 slabs of 512KB, deferred past the latency-
                # critical early window and done well ahead of phase C.
                fc2 = fmg - 8
                eng = [nc.sync, nc.scalar][fc2 % 2]
                eng.dma_start(
                    out=wd_sb[:, 2 * fc2 : 2 * fc2 + 2],
                    in_=wd_d[:, 2 * fc2 : 2 * fc2 + 2],
                )
            for nb0, nbl in blocks:
                ps_g = psP.tile([P, NB], F32, tag="ps")
                ps_u = psP.tile([P, NB], F32, tag="ps")
                for i, dc in enumerate(DC_ORDER):
                    nc.tensor.matmul(
                        ps_g[:, :nbl],
                        lhsT=wg_sb[:, dc, :],
                        rhs=x_sb[:, dc, nb0 : nb0 + nbl],
                        start=(i == 0),
                        stop=(i == DC - 1),
                    )
                for i, dc in enumerate(DC_ORDER):
                    nc.tensor.matmul(
                        ps_u[:, :nbl],
                        lhsT=wu_sb[:, dc, :],
                        rhs=x_sb[:, dc, nb0 : nb0 + nbl],
                        start=(i == 0),
                        stop=(i == DC - 1),
                    )
                # Both drain ops on the vector engine: the scalar engine's
                # stream carries pool-paced DMA triggers whose waits would
                # head-of-line-block a relu placed behind them.
                g_sb = gopool.tile([P, NB], F32, tag="g")
                nc.vector.tensor_scalar_max(g_sb[:, :nbl], ps_g[:, :nbl], 0.0)
                nc.vector.tensor_mul(
                    h_all[:, fl, off + nb0 : off + nb0 + nbl],
                    g_sb[:, :nbl],
                    ps_u[:, :nbl],
                )

    # ---- Phase C: part[d, t] = WdT.T @ h, Wd stationary / h moving ----
    # Per (slot, token block, dn): one PSUM bank accumulates 8 back-to-back
    # matmuls; drains and output DMAs rotate across engines / queues. The
    # slow SWDGE (gpsimd) queue only gets every 5th output DMA (the doubled
    # output volume would otherwise outlast phase C on it), and none of the
    # last 2 blocks', so its end-of-program queue drain doesn't extend the
    # tail. The final block is the narrowest and its last drain runs on
    # scalar (the vector queue carries a semaphore backlog at the end).
    qi = 0
    units = []
    for s in reversed(range(QS)):
        off, W = slots[s]
        for nb0, nbl in _nblocks(W):
            units.append((s, off, nb0, nbl))
    nunits = len(units)
    for ui, (s, off, nb0, nbl) in enumerate(units):
        late = ui >= nunits - 2
        for dn in range(DN):
            ps_o = psP.tile([P, NB], F32, tag="ps")
            for fl in range(QFM):
                nc.tensor.matmul(
                    ps_o[:, :nbl],
                    lhsT=wd_sb[:, s * QFM + fl, dn, :],
                    rhs=h_all[:, fl, off + nb0 : off + nb0 + nbl],
                    start=(fl == 0),
                    stop=(fl == QFM - 1),
                )
            o_sb = gopool.tile([P, NB], F16, tag="o")
            # each unit's output DMA rides the queue of the engine that
            # drains it (scalar drain -> scalar queue, vector drain -> sync
            # queue): the DMA's read-after-drain wait is then satisfied in
            # program order and never head-of-line-blocks another engine's
            # drains. The slow SWDGE (gpsimd) queue gets every 5th unit,
            # none in the last 2 blocks.
            par = (dn % 2 == 0) if ui < nunits - 1 else (dn % 2 == 1)
            if par:
                nc.scalar.activation(
                    out=o_sb[:, :nbl],
                    in_=ps_o[:, :nbl],
                    func=mybir.ActivationFunctionType.Copy,
                )
            else:
                nc.vector.tensor_scalar_mul(o_sb[:, :nbl], ps_o[:, :nbl], 1.0)
            if not late and qi % 5 == 4:
                q = nc.gpsimd
            else:
                q = nc.scalar if par else nc.sync
            q.dma_start(
                out=out_d[dn, :, off + nb0 : off + nb0 + nbl],
                in_=o_sb[:, :nbl],
            )
            qi += 1


_NC_CACHE: dict = {}


def _get_nc(widths):
    if widths not in _NC_CACHE:
        _NC_CACHE[widths] = _build_nc(widths)
    return _NC_CACHE[widths]


def _round_tf32(a):
    """Round-to-nearest-even fp32 -> TF32 (10-bit mantissa), as np.float32."""
    u = a.astype(np.float32).view(np.uint32).astype(np.uint64)
    lsb = (u >> 13) & 1
    r = (u + 0x0FFF + lsb) & 0xFFFFE000
    return r.astype(np.uint32).view(np.float32)


def _mm_round(a):
    """Convert a host array to the dtype/value the device matmuls consume."""
    if MM_DT is F32R:
        return _round_tf32(a)
    if MM_DT is F16:
        return np.ascontiguousarray(a, dtype=np.float16)
    return np.ascontiguousarray(a, dtype=np.float32)


def _route_host(x, Wr):
    """Reference-identical routing on host (jax on CPU, same ops as reference).

    Returns (k_ids [T, K] int, k_w [T, K] f32).
    """
    import jax
    import jax.numpy as jnp

    cpu = jax.devices("cpu")[0]
    with jax.default_device(cpu):
        xt = jnp.asarray(x.reshape(T, D))
        logits = jnp.einsum("td,ed->te", xt, jnp.asarray(Wr))
        scores = jax.nn.softmax(logits, axis=-1)
        k_scores, k_ids = jax.lax.top_k(scores, TOPK)
        eps = jnp.finfo(x.dtype).eps
        k_w = k_scores / (k_scores.sum(axis=-1, keepdims=True) + eps)
        return np.asarray(k_ids), np.asarray(k_w)


def _prep_weights(Wg, Wu, Wd):
    """Per-expert weight tensors in device layouts (contiguous, rounded)."""
    wg_r, wu_r, wd_r = [], [], []
    for e in range(len(Wg)):
        # Wg[e]: [DFF, D]; device wants [fm, p(d_inner), dc, f_inner]
        wgt = Wg[e].T.reshape(DC, P, FM, P).transpose(2, 1, 0, 3)
        wut = Wu[e].T.reshape(DC, P, FM, P).transpose(2, 1, 0, 3)
        # Wd[e]: [D, DFF]; device wants [p(f_inner), fc, dn, d_inner]
        wdt = Wd[e].reshape(DN, P, FM, P).transpose(3, 2, 0, 1)
        wg_r.append(_mm_round(np.ascontiguousarray(wgt, dtype=np.float32)))
        wu_r.append(_mm_round(np.ascontiguousarray(wut, dtype=np.float32)))
        wd_r.append(_mm_round(np.ascontiguousarray(wdt, dtype=np.float32)))
    return wg_r, wu_r, wd_r


def kernel(x, Wr, Wg, Wu, Wd):
    global LAST_EXEC_NS
    x = np.asarray(x, dtype=np.float32)
    Wr = np.asarray(Wr, dtype=np.float32)
    Wg = np.asarray(Wg, dtype=np.float32)
    Wu = np.asarray(Wu, dtype=np.float32)
    Wd = np.asarray(Wd, dtype=np.float32)

    k_ids, k_w = _route_host(x, Wr)
    xt = x.reshape(T, D)

    # Gather per-expert token lists (each token appears once per selected expert).
    idx_lists, w_lists = [], []
    for e in range(E):
        tmask = k_ids == e                       # [T, K]
        tok = np.nonzero(tmask.any(axis=1))[0]   # unique tokens routed to e
        wvals = (k_w * tmask).sum(axis=1)[tok].astype(np.float32)
        idx_lists.append(tok)
        w_lists.append(wvals)

    loads = np.array([len(t) for t in idx_lists])
    # Rank experts by load; slot position k hosts experts ranked 2k (cores
    # 0-3) and 2k+1 (cores 4-7), one DFF-quarter per core, so slot width k
    # only pads to the rank-2k load instead of the global max.
    order = [int(e) for e in np.argsort(-loads, kind="stable")]
    widths = tuple(
        max(P, ((int(loads[order[2 * k]]) + 7) // 8) * 8) for k in range(QS)
    )
    offs = [sum(widths[:s]) for s in range(QS)]
    Tc = sum(widths)
    assert Tc <= 4600 and max(widths) <= 1400, f"imbalanced routing ({widths})"

    wg_r, wu_r, wd_r = _prep_weights(Wg, Wu, Wd)

    in_maps = []
    for c in range(NCORES):
        q = c % 4
        xg = np.zeros((Tc, D), dtype=np.float32)
        wg_t, wu_t, wd_t = [], [], []
        for k in range(QS):
            e = order[2 * k] if c < 4 else order[2 * k + 1]
            xg[offs[k] : offs[k] + loads[e]] = xt[idx_lists[e]]
            sl = slice(q * QFM, (q + 1) * QFM)
            wg_t.append(wg_r[e][sl])
            wu_t.append(wu_r[e][sl])
            wd_t.append(wd_r[e][:, sl])
        xg_r = _mm_round(
            np.ascontiguousarray(
                xg.T.reshape(DC, P, Tc).transpose(1, 0, 2), dtype=np.float32
            )
        )
        in_maps.append(
            {
                "x": xg_r,
                "wg": np.ascontiguousarray(np.concatenate(wg_t, axis=0)),
                "wu": np.ascontiguousarray(np.concatenate(wu_t, axis=0)),
                "wd": np.ascontiguousarray(np.concatenate(wd_t, axis=1)),
            }
        )

    nc = _get_nc(widths)
    core_ids = list(range(NCORES))
    if PROFILE:
        res = _run_profiled(nc, in_maps, core_ids)
        LAST_EXEC_NS = res.exec_time_ns
        results = res.results
    else:
        results = run_bass_kernel_spmd(nc, in_maps, core_ids).results

    out = np.zeros((T, D), dtype=np.float32)
    outs = [results[c]["out"].reshape(D, Tc).astype(np.float32) for c in range(NCORES)]
    for k in range(QS):
        for group, e in ((range(0, 4), order[2 * k]), (range(4, 8), order[2 * k + 1])):
            # the four quarter-cores' partials sum to the full GluMLP output
            # (exact fp32 adds)
            oe = sum(outs[c][:, offs[k] : offs[k] + loads[e]] for c in group)
            out[idx_lists[e]] += w_lists[e][:, None] * oe.T
    return out.reshape(B, L, D)


def _run_profiled(nc, in_maps, core_ids):
    """run_bass_kernel_spmd with trace=True, providing the NTFF hook that the
    agent image's antenv stub lacks, and skipping the artifact upload."""
    import sys
    import tempfile
    import types

    import concourse.bass_utils as bu

    if "antenv.axon_hooks" not in sys.modules:
        from trn_agent_boot.trn_boot import _ntff_profile_via_ctypes

        hook = _ntff_profile_via_ctypes("/opt/axon/libaxon_pjrt.so")
        mod = types.ModuleType("antenv.axon_hooks")
        mod.get_axon_ntff_profile_hook = lambda: hook
        mod.set_axon_ntff_profile_hook = lambda h: None
        sys.modules["antenv.axon_hooks"] = mod

    orig_upload = bu.upload_artifacts
    bu.upload_artifacts = lambda tmpdir: ""
    try:
        return run_bass_kernel_spmd(
            nc,
            in_maps,
            core_ids,
            trace=True,
            trace_cores=TRACE_CORES,
            tmpdir=tempfile.mkdtemp(prefix="moe_ntff_"),
        )
    finally:
        bu.upload_artifacts = orig_upload


if __name__ == "__main__":
    # smoke test with random data (no reference comparison)
    rng = np.random.default_rng(0)
    ins = {
        "x": rng.standard_normal((B, L, D), dtype=np.float32),
        "Wr": (rng.standard_normal((E, D)) * 0.02).astype(np.float32),
        "Wg": (rng.standard_normal((E, DFF, D)) * 0.02).astype(np.float32),
        "Wu": (rng.standard_normal((E, DFF, D)) * 0.02).astype(np.float32),
        "Wd": (rng.standard_normal((E, D, DFF)) * 0.02).astype(np.float32),
    }
    out = kernel(**ins)
    print("out", out.shape, out.dtype, float(np.abs(out).max()))
